# revision 13
# baseline (speedup 1.0000x reference)
"""Trainium2 Bass kernel for nn_MoELLMMini (2-layer MoE transformer + vocab head).

Sharding over 8 NeuronCores (SPMD, one program, per-core data):
  - activations replicated, resident TRANSPOSED in SBUF: xT [D=512, N=4096] fp32
  - attention sharded BY HEAD (core c owns head c; q/k/v/o weight slices are
    per-core input data), partial outputs combined with f16 AllReduce
  - MoE expert-parallel (core c owns expert c): on-device top-2 routing,
    compaction via indirect-DMA scatter, FFN on <=1280 tokens, scatter-back,
    f16 AllReduce combine
  - head vocab-sharded: core c computes logits[:, 4000c:4000(c+1)], host concats
Precision: fp16 matmuls (routing-stable vs fp32 reference), fp32 gate scores,
fp32 LN/residual stream, fp32 output.
"""
import numpy as np

import concourse.bass as bass
import concourse.mybir as mybir
import concourse.tile as tile
from concourse.bass_utils import run_bass_kernel_spmd
from concourse.masks import make_upper_triangular, make_identity
from concourse.vector_clock import ScopedClock, VectorClock

FP32 = mybir.dt.float32
F16 = mybir.dt.float16
I32 = mybir.dt.int32
AF = mybir.ActivationFunctionType
ALU = mybir.AluOpType

P = 128
D = 512
DC = 4
FF = 2048
FC = 16
NTOK = 4096
NCH = 32
NSLAB = 8
E = 8
HD = 64
CAP = 1280
CCH = 10
VSH = 4000
NCORE = 8
EPS = 1e-5
DUMP_TOK = float(NTOK)


def _patched_drain_and_barrier(self, tick_clock, wait_clock):
    gcl = tick_clock.global_clock
    n = len(gcl)
    for proc in range(n):
        t = gcl[proc]
        if t > 0:
            vec = [0] * n
            vec[proc] = t
            nop = self.nc.sync.nop(nofuse=True)
            wait_clock.add_sem_waits(nop.ins, ScopedClock({None: VectorClock(vec)}))
    self.nc.sync.drain()
    self.nc.all_engine_barrier()
    popped = self.nc._tile_sem_poison_stack.pop()
    assert popped is self._sem_poison
    self.nc.clear_and_free_semaphores(list(self.sems.allocated().values()))
    self.nc.all_engine_barrier()


tile.TileContext._drain_and_barrier = _patched_drain_and_barrier


def _split_multi_waits(nc, maxw=1):
    for f in nc.m.functions:
        for bb in f.blocks:
            old = list(bb.instructions)
            new = []
            for ins in old:
                si = ins.sync_info
                if si is not None and len(si.on_wait) > maxw:
                    waits = list(si.on_wait)
                    hoist, keep = waits[:-maxw], waits[-maxw:]
                    for w in hoist:
                        nop = mybir.InstNoOp(
                            name=nc.get_next_instruction_name(), ins=[], outs=[]
                        )
                        nop.engine = ins.engine
                        nop.sync_info = mybir.SyncInfo(on_wait=[w], on_update=[])
                        new.append(nop)
                    ins.sync_info = mybir.SyncInfo(
                        on_wait=keep, on_update=list(si.on_update)
                    )
                new.append(ins)
            bb.instructions = new


def build_kernel():
    nc = bass.Bass()
    dp = nc.declare_dram_parameter

    x0T = dp("x0T", [D, NTOK], FP32, isOutput=False)
    L = []
    for l in range(2):
        L.append(dict(
            wq=dp(f"l{l}_wq", [D, HD], F16, isOutput=False),
            bq=dp(f"l{l}_bq", [HD, 1], FP32, isOutput=False),
            wk=dp(f"l{l}_wk", [D, HD], F16, isOutput=False),
            bk=dp(f"l{l}_bk", [HD, 1], FP32, isOutput=False),
            wv=dp(f"l{l}_wv", [D, HD], F16, isOutput=False),
            bv=dp(f"l{l}_bv", [1, HD], F16, isOutput=False),
            ow=dp(f"l{l}_ow", [HD, D], F16, isOutput=False),
            ob8=dp(f"l{l}_ob8", [P, DC], FP32, isOutput=False),
            n1g=dp(f"l{l}_n1g", [P, DC], FP32, isOutput=False),
            n1b=dp(f"l{l}_n1b", [P, DC], FP32, isOutput=False),
            n2g=dp(f"l{l}_n2g", [P, DC], FP32, isOutput=False),
            n2b=dp(f"l{l}_n2b", [P, DC], FP32, isOutput=False),
            gw=dp(f"l{l}_gw", [D, E], FP32, isOutput=False),
            gb=dp(f"l{l}_gb", [1, E], FP32, isOutput=False),
            esel=dp(f"l{l}_esel", [1, E], FP32, isOutput=False),
            w1=dp(f"l{l}_w1", [D, FF], F16, isOutput=False),
            b1=dp(f"l{l}_b1", [P, FC], FP32, isOutput=False),
            w2=dp(f"l{l}_w2", [FF, D], F16, isOutput=False),
            b2=dp(f"l{l}_b2", [P, DC], FP32, isOutput=False),
        ))
    lfg = dp("lfg", [P, DC], FP32, isOutput=False)
    lfb = dp("lfb", [P, DC], FP32, isOutput=False)
    hw = dp("hw", [D, VSH], F16, isOutput=False)
    hb = dp("hb", [1, VSH], F16, isOutput=False)
    out = dp("out", [NTOK, VSH], FP32, isOutput=True)

    with tile.TileContext(nc) as tc:
        with (
            tc.tile_pool(name="const", bufs=1) as cpool,
            tc.tile_pool(name="resident", bufs=1) as rpool,
            tc.tile_pool(name="dram", bufs=1, space="DRAM") as dram,
        ):
            # ---- constants
            ones_col_f32 = cpool.tile([P, 1], FP32)
            nc.vector.memset(ones_col_f32[:], 1.0)
            ones_col_f16 = cpool.tile([P, 1], F16)
            nc.vector.memset(ones_col_f16[:], 1.0)
            ones_row_f32 = cpool.tile([1, P], FP32)
            nc.vector.memset(ones_row_f32[:], 1.0)
            ones_row_f16 = cpool.tile([1, P], F16)
            nc.vector.memset(ones_row_f16[:], 1.0)
            utri = cpool.tile([P, P], FP32)
            make_upper_triangular(nc, utri[:], val=1.0, diag=True)
            ident = cpool.tile([P, P], FP32)
            make_identity(nc, ident[:])
            zrow16 = cpool.tile([P, 516], F16)
            nc.vector.memset(zrow16[:], 0.0)
            iota_p16 = cpool.tile([P, 1], F16)
            it2 = cpool.tile([P, 1], I32)
            nc.gpsimd.iota(it2[:], pattern=[[0, 1]], base=0, channel_multiplier=1)
            nc.vector.tensor_copy(iota_p16[:], it2[:])
            dump_cap = cpool.tile([P, NCH], FP32)
            nc.vector.memset(dump_cap[:], float(CAP))
            eps_t = cpool.tile([P, 1], FP32)
            nc.vector.memset(eps_t[:], EPS)

            # ---- resident activations
            xT = rpool.tile([P, DC, NTOK], FP32)
            xT16 = rpool.tile([P, DC, NTOK], F16)
            nc.sync.dma_start(xT[:], x0T.rearrange("(c p) t -> p c t", p=P))
            nc.vector.tensor_copy(xT16[:], xT[:])

            # ---- dram scratch
            arA_in = dram.tile([D, NTOK], F16)
            arA_outs = [
                dram.tile([D, NTOK], F16, addr_space="Shared", tag="aro0", name="aro0"),
                dram.tile([D, NTOK], F16, addr_space="Shared", tag="aro1", name="aro1"),
            ]
            xT16_d = dram.tile([D, NTOK], F16)
            xe_d = dram.tile([CAP + 1, 516], F16)
            wrow_d = dram.tile([1, CAP], FP32)
            ye_d = dram.tile([D, CAP], F16)
            Y_d = dram.tile([NTOK + 1, D], F16)
            Yars = [
                dram.tile([NTOK + 1, D], F16, addr_space="Shared", tag="yar0", name="yar0"),
                dram.tile([NTOK + 1, D], F16, addr_space="Shared", tag="yar1", name="yar1"),
            ]
            RG = [list(range(NCORE))]

            def layernorm(g_dram, b_dram):
                with (
                    tc.tile_pool(name="lnp", bufs=2) as pool,
                    tc.tile_pool(name="lnps", bufs=2, space="PSUM") as psB,
                ):
                    gam = pool.tile([P, DC], FP32, tag="lng")
                    bet = pool.tile([P, DC], FP32, tag="lnb")
                    nc.sync.dma_start(gam[:], g_dram[:])
                    nc.sync.dma_start(bet[:], b_dram[:])
                    for s in range(NSLAB):
                        sl = slice(512 * s, 512 * (s + 1))
                        xs = xT[:, :, sl]
                        msum = psB.tile([1, 512], FP32, tag="stat")
                        for c in range(DC):
                            nc.tensor.matmul(msum[:], lhsT=ones_col_f32[:],
                                             rhs=xs[:, c],
                                             start=(c == 0), stop=(c == DC - 1))
                        vsum = psB.tile([1, 512], FP32, tag="stat")
                        for c in range(DC):
                            sq = pool.tile([P, 512], FP32, tag="ln_sq")
                            nc.vector.tensor_tensor(sq[:], xs[:, c], xs[:, c],
                                                    op=ALU.mult)
                            nc.tensor.matmul(vsum[:], lhsT=ones_col_f32[:],
                                             rhs=sq[:],
                                             start=(c == 0), stop=(c == DC - 1))
                        m = pool.tile([1, 512], FP32, tag="ln_m")
                        nc.vector.tensor_scalar_mul(m[:], msum[:], 1.0 / D)
                        var = pool.tile([1, 512], FP32, tag="ln_var")
                        nc.vector.tensor_tensor(var[:], m[:], m[:], op=ALU.mult)
                        ex2 = pool.tile([1, 512], FP32, tag="ln_e2")
                        nc.vector.tensor_scalar_mul(ex2[:], vsum[:], 1.0 / D)
                        nc.vector.tensor_tensor(var[:], ex2[:], var[:],
                                                op=ALU.subtract)
                        sd = pool.tile([1, 512], FP32, tag="ln_sd")
                        nc.scalar.activation(sd[:], var[:], AF.Sqrt,
                                             bias=eps_t[0:1, 0:1])
                        inv = pool.tile([1, 512], FP32, tag="ln_inv")
                        nc.vector.reciprocal(inv[:], sd[:])
                        mb = psB.tile([P, 512], FP32, tag="bc")
                        nc.tensor.matmul(mb[:], lhsT=ones_row_f32[:], rhs=m[:],
                                         start=True, stop=True)
                        ib = psB.tile([P, 512], FP32, tag="bc")
                        nc.tensor.matmul(ib[:], lhsT=ones_row_f32[:], rhs=inv[:],
                                         start=True, stop=True)
                        for c in range(DC):
                            t1 = pool.tile([P, 512], FP32, tag="ln_t1")
                            nc.vector.tensor_tensor(t1[:], xs[:, c], mb[:],
                                                    op=ALU.subtract)
                            nc.vector.tensor_tensor(t1[:], t1[:], ib[:],
                                                    op=ALU.mult)
                            nc.scalar.activation(xs[:, c], t1[:], AF.Identity,
                                                 bias=bet[:, c:c + 1],
                                                 scale=gam[:, c:c + 1])
                        nc.vector.tensor_copy(xT16[:, :, sl], xs)

            # ================= layers =================
            for l in range(2):
                lp = L[l]
                arA_out = arA_outs[l]
                Yar = Yars[l]

                # ---------- attention (this core's head) ----------
                with (
                    tc.tile_pool(name="ap", bufs=2) as pool,
                    tc.tile_pool(name="abig", bufs=1) as bigpool,
                    tc.tile_pool(name="aex", bufs=2) as hpool,
                    tc.tile_pool(name="aps", bufs=3, space="PSUM") as psum,
                    tc.tile_pool(name="apsb", bufs=2, space="PSUM") as psB,
                ):
                    wq = pool.tile([P, DC, HD], F16, tag="wq")
                    nc.sync.dma_start(
                        wq[:], lp["wq"].rearrange("(c p) f -> p c f", p=P))
                    bq = pool.tile([HD, 1], FP32, tag="bq")
                    nc.sync.dma_start(bq[:], lp["bq"][:])
                    wk = pool.tile([P, DC, HD], F16, tag="wk")
                    nc.sync.dma_start(
                        wk[:], lp["wk"].rearrange("(c p) f -> p c f", p=P))
                    bk = pool.tile([HD, 1], FP32, tag="bk")
                    nc.sync.dma_start(bk[:], lp["bk"][:])
                    wv = pool.tile([P, DC, HD], F16, tag="wv")
                    nc.sync.dma_start(
                        wv[:], lp["wv"].rearrange("(c p) f -> p c f", p=P))
                    bvr = pool.tile([1, HD], F16, tag="bvr")
                    nc.sync.dma_start(bvr[:], lp["bv"][:])
                    oww = pool.tile([HD, D], F16, tag="oww")
                    nc.sync.dma_start(oww[:], lp["ow"][:])
                    ob8 = pool.tile([P, DC], FP32, tag="ob8")
                    nc.sync.dma_start(ob8[:], lp["ob8"][:])

                    vb_ps = psB.tile([P, 512], FP32, tag="small")
                    nc.tensor.matmul(vb_ps[:, :HD], lhsT=ones_row_f16[:],
                                     rhs=bvr[:], start=True, stop=True)
                    vb = pool.tile([P, HD], F16, tag="vb")
                    nc.vector.tensor_copy(vb[:], vb_ps[:, :HD])

                    qT = bigpool.tile([HD, NTOK], F16, tag="qT")
                    kT = bigpool.tile([HD, NTOK], F16, tag="kT")
                    for s in range(NSLAB):
                        sl = slice(512 * s, 512 * (s + 1))
                        for wmat, bvec, dst in ((wq, bq, qT), (wk, bk, kT)):
                            ps = psum.tile([P, 512], FP32, tag="mm")
                            for c in range(DC):
                                nc.tensor.matmul(ps[:HD, :], lhsT=wmat[:, c, :],
                                                 rhs=xT16[:, c, sl],
                                                 start=(c == 0),
                                                 stop=(c == DC - 1))
                            nc.scalar.activation(dst[:, sl], ps[:HD, :],
                                                 AF.Identity, bias=bvec[:, 0:1])
                    V = bigpool.tile([P, NCH, HD], F16, tag="V")
                    for t in range(NCH):
                        ps = psB.tile([P, 512], FP32, tag="small")
                        for c in range(DC):
                            nc.tensor.matmul(
                                ps[:, :HD],
                                lhsT=xT16[:, c, 128 * t:128 * (t + 1)],
                                rhs=wv[:, c, :],
                                start=(c == 0), stop=(c == DC - 1))
                        nc.vector.tensor_tensor(V[:, t, :], ps[:, :HD], vb[:],
                                                op=ALU.add)

                    attVT = bigpool.tile([HD, NTOK], F16, tag="attVT")
                    for b in range(2):
                        for qs in range(4):
                            qsl = slice(2048 * b + 512 * qs,
                                        2048 * b + 512 * (qs + 1))
                            expT = hpool.tile([P, 16, 512], F16, tag="expT")
                            for kc in range(16):
                                kg = 16 * b + kc
                                ps = psum.tile([P, 512], FP32, tag="mm")
                                nc.tensor.matmul(
                                    ps[:],
                                    lhsT=kT[:, 128 * kg:128 * (kg + 1)],
                                    rhs=qT[:, qsl], start=True, stop=True)
                                nc.scalar.activation(expT[:, kc, :], ps[:],
                                                     AF.Exp, scale=0.125)
                            dn = psB.tile([1, 512], FP32, tag="dn")
                            for kc in range(16):
                                nc.tensor.matmul(dn[:], lhsT=ones_col_f16[:],
                                                 rhs=expT[:, kc, :],
                                                 start=(kc == 0), stop=(kc == 15))
                            rden = pool.tile([1, 512], FP32, tag="rden")
                            nc.vector.reciprocal(rden[:], dn[:])
                            rb_ps = psB.tile([P, 512], FP32, tag="small")
                            nc.tensor.matmul(rb_ps[:HD, :],
                                             lhsT=ones_row_f32[:, :HD],
                                             rhs=rden[:], start=True, stop=True)
                            rbc = pool.tile([HD, 512], FP32, tag="rbc")
                            nc.vector.tensor_copy(rbc[:], rb_ps[:HD, :])
                            av = psum.tile([P, 512], FP32, tag="mm")
                            for kc in range(16):
                                kg = 16 * b + kc
                                nc.tensor.matmul(av[:HD, :], lhsT=V[:, kg, :],
                                                 rhs=expT[:, kc, :],
                                                 start=(kc == 0), stop=(kc == 15))
                            nc.vector.tensor_tensor(attVT[:, qsl], av[:HD, :],
                                                    rbc[:], op=ALU.mult)

                    for s in range(NSLAB):
                        sl = slice(512 * s, 512 * (s + 1))
                        ao = pool.tile([P, DC, 512], F16, tag="ao")
                        for c in range(DC):
                            ps = psum.tile([P, 512], FP32, tag="mm")
                            nc.tensor.matmul(
                                ps[:], lhsT=oww[:, 128 * c:128 * (c + 1)],
                                rhs=attVT[:, sl], start=True, stop=True)
                            nc.scalar.activation(ao[:, c], ps[:], AF.Identity,
                                                 bias=ob8[:, c:c + 1])
                        nc.sync.dma_start(
                            arA_in.rearrange("(c p) t -> p c t", p=P)[:, :, sl],
                            ao[:])
                    nc.gpsimd.collective_compute(
                        "AllReduce", ALU.add, replica_groups=RG,
                        ins=[arA_in.opt()], outs=[arA_out.opt()])
                    for s in range(NSLAB):
                        sl = slice(512 * s, 512 * (s + 1))
                        r16 = pool.tile([P, DC, 512], F16, tag="r16")
                        nc.sync.dma_start(
                            r16[:],
                            arA_out.rearrange("(c p) t -> p c t", p=P)[:, :, sl])
                        r32 = pool.tile([P, DC, 512], FP32, tag="r32")
                        nc.vector.tensor_copy(r32[:], r16[:])
                        nc.vector.tensor_tensor(xT[:, :, sl], xT[:, :, sl],
                                                r32[:], op=ALU.add)

                layernorm(lp["n1g"], lp["n1b"])

                # ---------- MoE ----------
                with (
                    tc.tile_pool(name="mp", bufs=2) as pool,
                    tc.tile_pool(name="mroute", bufs=1) as rt,
                    tc.tile_pool(name="mbig", bufs=1) as bigpool,
                    tc.tile_pool(name="mps", bufs=3, space="PSUM") as psum,
                    tc.tile_pool(name="mpsb", bufs=2, space="PSUM") as psB,
                ):
                    gw = pool.tile([P, DC, E], FP32, tag="gw")
                    nc.sync.dma_start(
                        gw[:], lp["gw"].rearrange("(c p) e -> p c e", p=P))
                    gb = pool.tile([1, E], FP32, tag="gb")
                    nc.sync.dma_start(gb[:], lp["gb"][:])
                    esel = pool.tile([1, E], FP32, tag="esel")
                    nc.sync.dma_start(esel[:], lp["esel"][:])
                    es_ps = psB.tile([P, E], FP32, tag="small")
                    nc.tensor.matmul(es_ps[:], lhsT=ones_row_f32[:], rhs=esel[:],
                                     start=True, stop=True)
                    eselb = rt.tile([P, E], FP32, tag="eselb")
                    nc.vector.tensor_copy(eselb[:], es_ps[:])

                    gate = rt.tile([P, NCH, E], FP32, tag="gate")
                    for t in range(NCH):
                        ps = psB.tile([P, E], FP32, tag="small")
                        for c in range(DC):
                            nc.tensor.matmul(
                                ps[:], lhsT=xT[:, c, 128 * t:128 * (t + 1)],
                                rhs=gw[:, c, :], start=(c == 0), stop=False)
                        nc.tensor.matmul(ps[:], lhsT=ones_row_f32[:], rhs=gb[:],
                                         start=False, stop=True)
                        nc.vector.tensor_copy(gate[:, t, :], ps[:])

                    def rtile(tag, dt=FP32):
                        return rt.tile([P, NCH], dt, tag=tag, name=tag)

                    mx = rt.tile([P, NCH, 8], FP32, tag="mx")
                    for t in range(NCH):
                        nc.vector.max(mx[:, t, :], gate[:, t, :])
                    sc1 = mx[:, :, 0]
                    sc2 = mx[:, :, 1]
                    d21 = rtile("d21")
                    nc.vector.tensor_tensor(d21[:], sc2, sc1, op=ALU.subtract)
                    e2 = rtile("e2")
                    nc.scalar.activation(e2[:], d21[:], AF.Exp)
                    Z = rtile("Z")
                    nc.vector.tensor_scalar(Z[:], e2[:], 1.0, None, op0=ALU.add)
                    p1 = rtile("p1")
                    nc.vector.reciprocal(p1[:], Z[:])
                    p2 = rtile("p2")
                    nc.vector.tensor_tensor(p2[:], e2[:], p1[:], op=ALU.mult)
                    gsel = rt.tile([P, NCH, E], FP32, tag="gsel")
                    nc.vector.tensor_tensor(
                        gsel[:], gate[:],
                        eselb[:, None, :].to_broadcast([P, NCH, E]), op=ALU.mult)
                    g4 = rt.tile([P, NCH, 4], FP32, tag="g4")
                    nc.vector.tensor_tensor(g4[:], gsel[:, :, 0:4],
                                            gsel[:, :, 4:8], op=ALU.add)
                    g2 = rt.tile([P, NCH, 2], FP32, tag="g2")
                    nc.vector.tensor_tensor(g2[:], g4[:, :, 0:2], g4[:, :, 2:4],
                                            op=ALU.add)
                    ge = rtile("ge")
                    nc.vector.tensor_tensor(ge[:], g2[:, :, 0], g2[:, :, 1],
                                            op=ALU.add)
                    m1 = rtile("m1")
                    nc.vector.tensor_tensor(m1[:], ge[:], sc1, op=ALU.is_equal)
                    m2 = rtile("m2")
                    nc.vector.tensor_tensor(m2[:], ge[:], sc2, op=ALU.is_equal)
                    nm1 = rtile("nm1")
                    nc.vector.tensor_scalar(nm1[:], m1[:], -1.0, 1.0,
                                            op0=ALU.mult, op1=ALU.add)
                    nc.vector.tensor_tensor(m2[:], m2[:], nm1[:], op=ALU.mult)
                    wme = rtile("wme")
                    nc.vector.tensor_tensor(wme[:], p1[:], m1[:], op=ALU.mult)
                    t6 = rtile("t6")
                    nc.vector.tensor_tensor(t6[:], p2[:], m2[:], op=ALU.mult)
                    nc.vector.tensor_tensor(wme[:], wme[:], t6[:], op=ALU.add)
                    mk = rtile("mk")
                    nc.vector.tensor_scalar(mk[:], wme[:], 0.0, None,
                                            op0=ALU.is_gt)
                    cum = psB.tile([P, NCH], FP32, tag="small")
                    nc.tensor.matmul(cum[:], lhsT=utri[:], rhs=mk[:],
                                     start=True, stop=True)
                    tot_ps = psB.tile([1, NCH], FP32, tag="small")
                    nc.tensor.matmul(tot_ps[:], lhsT=ones_col_f32[:], rhs=mk[:],
                                     start=True, stop=True)
                    tot = rt.tile([1, NCH], FP32, tag="tot")
                    nc.vector.tensor_copy(tot[:], tot_ps[:])
                    inc = rt.tile([1, NCH], FP32, tag="inc")
                    nc.vector.tensor_tensor_scan(inc[:], tot[:], tot[:], 0.0,
                                                 op0=ALU.add, op1=ALU.bypass)
                    exc = rt.tile([1, NCH], FP32, tag="exc")
                    nc.vector.tensor_tensor(exc[:], inc[:], tot[:],
                                            op=ALU.subtract)
                    offs = psB.tile([P, NCH], FP32, tag="small")
                    nc.tensor.matmul(offs[:], lhsT=ones_row_f32[:], rhs=exc[:],
                                     start=True, stop=True)
                    pos = rtile("pos")
                    nc.vector.tensor_tensor(pos[:], cum[:], mk[:],
                                            op=ALU.subtract)
                    nc.vector.tensor_tensor(pos[:], pos[:], offs[:], op=ALU.add)
                    nmk = rtile("nmk")
                    nc.vector.tensor_scalar(nmk[:], mk[:], -float(CAP),
                                            float(CAP), op0=ALU.mult,
                                            op1=ALU.add)
                    nc.vector.tensor_tensor(pos[:], pos[:], mk[:], op=ALU.mult)
                    nc.vector.tensor_tensor(pos[:], pos[:], nmk[:], op=ALU.add)
                    gtm = rt.tile([P, NCH], mybir.dt.uint32, tag="gtm",
                                  name="gtm")
                    nc.vector.tensor_scalar(gtm[:], pos[:], float(CAP), None,
                                            op0=ALU.is_ge)
                    nc.vector.copy_predicated(pos[:], gtm[:], dump_cap[:])
                    posi = rt.tile([P, NCH], I32, tag="posi")
                    nc.vector.tensor_copy(posi[:], pos[:])

                    whi = rt.tile([P, NCH], F16, tag="whi")
                    nc.vector.tensor_copy(whi[:], wme[:])
                    whi32 = rtile("whi32")
                    nc.vector.tensor_copy(whi32[:], whi[:])
                    wlo = rtile("wlo")
                    nc.vector.tensor_tensor(wlo[:], wme[:], whi32[:],
                                            op=ALU.subtract)
                    wlo16 = rt.tile([P, NCH], F16, tag="wlo16")
                    nc.vector.tensor_copy(wlo16[:], wlo[:])

                    # token-major x via DMA transpose; compose + scatter
                    nc.sync.dma_start(
                        xT16_d.rearrange("(c p) t -> p c t", p=P), xT16[:])
                    for i in range(11):
                        rows = min(P, CAP + 1 - P * i)
                        nc.sync.dma_start(xe_d[P * i:P * i + rows, :],
                                          zrow16[:rows])
                    for s in range(NSLAB):
                        xtok = pool.tile([P, DC, D], F16, tag="xtok")
                        nc.sync.dma_start_transpose(
                            xtok[:], xT16_d[:, 512 * s:512 * (s + 1)])
                        for j in range(DC):
                            t = 4 * s + j
                            row = pool.tile([P, 516], F16, tag="crow")
                            nc.vector.tensor_copy(row[:, 0:D], xtok[:, j, :])
                            nc.vector.tensor_copy(row[:, D:D + 1],
                                                  whi[:, t:t + 1])
                            nc.vector.tensor_copy(row[:, D + 1:D + 2],
                                                  wlo16[:, t:t + 1])
                            nc.vector.memset(row[:, D + 2:D + 3], float(t))
                            nc.vector.tensor_copy(row[:, D + 3:D + 4],
                                                  iota_p16[:])
                            nc.gpsimd.indirect_dma_start(
                                out=xe_d[:],
                                out_offset=bass.IndirectOffsetOnAxis(
                                    ap=posi[:, t:t + 1], axis=0),
                                in_=row[:], in_offset=None)

                    xeT = bigpool.tile([P, DC, CAP], F16, tag="xeT")
                    nc.sync.dma_start_transpose(xeT[:], xe_d[0:CAP, 0:D])
                    wt = pool.tile([P, CCH, 4], F16, tag="wt")
                    nc.sync.dma_start(
                        wt[:],
                        xe_d[0:CAP, D:D + 4].rearrange("(j p) c -> p j c", p=P))
                    wa = pool.tile([P, CCH], FP32, tag="wa")
                    nc.vector.tensor_copy(wa[:], wt[:, :, 0])
                    wb = pool.tile([P, CCH], FP32, tag="wb")
                    nc.vector.tensor_copy(wb[:], wt[:, :, 1])
                    wcmp = rt.tile([P, CCH], FP32, tag="wcmp")
                    nc.vector.tensor_tensor(wcmp[:], wa[:], wb[:], op=ALU.add)
                    ja = pool.tile([P, CCH], FP32, tag="ja")
                    nc.vector.tensor_copy(ja[:], wt[:, :, 2])
                    pa = pool.tile([P, CCH], FP32, tag="pa")
                    nc.vector.tensor_copy(pa[:], wt[:, :, 3])
                    tok = pool.tile([P, CCH], FP32, tag="tok")
                    nc.vector.tensor_scalar(tok[:], ja[:], 128.0, None,
                                            op0=ALU.mult)
                    nc.vector.tensor_tensor(tok[:], tok[:], pa[:], op=ALU.add)
                    vld = pool.tile([P, CCH], FP32, tag="vld")
                    nc.vector.tensor_scalar(vld[:], wcmp[:], 0.0, None,
                                            op0=ALU.is_gt)
                    nvld = pool.tile([P, CCH], FP32, tag="nvld")
                    nc.vector.tensor_scalar(nvld[:], vld[:], -DUMP_TOK,
                                            DUMP_TOK, op0=ALU.mult, op1=ALU.add)
                    nc.vector.tensor_tensor(tok[:], tok[:], vld[:], op=ALU.mult)
                    nc.vector.tensor_tensor(tok[:], tok[:], nvld[:], op=ALU.add)
                    toki = rt.tile([P, CCH], I32, tag="toki")
                    nc.vector.tensor_copy(toki[:], tok[:])

                    # FFN
                    b1t = pool.tile([P, FC], FP32, tag="b1")
                    nc.sync.dma_start(b1t[:], lp["b1"][:])
                    b2t = pool.tile([P, DC], FP32, tag="b2")
                    nc.sync.dma_start(b2t[:], lp["b2"][:])

                    SLABS = [(0, 512), (512, 512), (1024, 256)]
                    w1r = lp["w1"].rearrange("(c p) f -> p c f", p=P)

                    wtr_ps = psB.tile([P, P], FP32, tag="small")
                    nc.tensor.transpose(wtr_ps[:CCH, :], wcmp[:], ident[:])
                    wtr = pool.tile([CCH, P], FP32, tag="wtrs")
                    nc.vector.tensor_copy(wtr[:], wtr_ps[:CCH, :])
                    nc.sync.dma_start(
                        wrow_d.rearrange("a (j p) -> (a j) p", p=P), wtr[:])
                    wrow = pool.tile([1, CAP], FP32, tag="wrow")
                    nc.sync.dma_start(wrow[:], wrow_d[:])
                    wbc = pool.tile([P, CAP], FP32, tag="wbc")
                    for s0, sw in SLABS:
                        wb_ps = psB.tile([P, 512], FP32, tag="small")
                        nc.tensor.matmul(wb_ps[:, :sw], lhsT=ones_row_f32[:],
                                         rhs=wrow[:, s0:s0 + sw],
                                         start=True, stop=True)
                        nc.vector.tensor_copy(wbc[:, s0:s0 + sw], wb_ps[:, :sw])

                    w2r = lp["w2"].rearrange("(c p) f -> p c f", p=P)
                    for s0, sw in SLABS:
                        hT = bigpool.tile([P, FC, 512], F16, tag="hT")
                        for fc in range(FC):
                            w1s = pool.tile([P, DC, P], F16, tag="w1s")
                            nc.sync.dma_start(
                                w1s[:], w1r[:, :, 128 * fc:128 * (fc + 1)])
                            ps = psum.tile([P, 512], FP32, tag="mm")
                            for c in range(DC):
                                nc.tensor.matmul(
                                    ps[:, :sw],
                                    lhsT=w1s[:, c, :],
                                    rhs=xeT[:, c, s0:s0 + sw],
                                    start=(c == 0), stop=(c == DC - 1))
                            nc.scalar.activation(hT[:, fc, :sw],
                                                 ps[:, :sw], AF.Relu,
                                                 bias=b1t[:, fc:fc + 1])
                        for dc in range(DC):
                            w2s = pool.tile([P, FC, P], F16, tag="w2s")
                            nc.sync.dma_start(
                                w2s[:], w2r[:, :, 128 * dc:128 * (dc + 1)])
                            ps = psum.tile([P, 512], FP32, tag="mm")
                            for fc in range(FC):
                                nc.tensor.matmul(
                                    ps[:, :sw],
                                    lhsT=w2s[:, fc, :],
                                    rhs=hT[:, fc, :sw],
                                    start=(fc == 0), stop=(fc == FC - 1))
                            yb = pool.tile([P, 512], FP32, tag="yb")
                            nc.scalar.activation(yb[:, :sw], ps[:, :sw],
                                                 AF.Identity,
                                                 bias=b2t[:, dc:dc + 1])
                            yes = pool.tile([P, 512], F16, tag="yes")
                            nc.vector.tensor_tensor(yes[:, :sw], yb[:, :sw],
                                                    wbc[:, s0:s0 + sw],
                                                    op=ALU.mult)
                            nc.sync.dma_start(
                                ye_d[128 * dc:128 * (dc + 1), s0:s0 + sw],
                                yes[:, :sw])

                    for i in range(33):
                        rows = min(P, NTOK + 1 - P * i)
                        nc.sync.dma_start(Y_d[P * i:P * i + rows, :],
                                          zrow16[:rows, 0:D])
                    for j in range(CCH):
                        yetok = pool.tile([P, D], F16, tag="yetok")
                        nc.sync.dma_start_transpose(
                            yetok[:], ye_d[:, 128 * j:128 * (j + 1)])
                        nc.gpsimd.indirect_dma_start(
                            out=Y_d[:],
                            out_offset=bass.IndirectOffsetOnAxis(
                                ap=toki[:, j:j + 1], axis=0),
                            in_=yetok[:], in_offset=None)
                    nc.gpsimd.collective_compute(
                        "AllReduce", ALU.add, replica_groups=RG,
                        ins=[Y_d.opt()], outs=[Yar.opt()])
                    for s in range(NSLAB):
                        sl = slice(512 * s, 512 * (s + 1))
                        yt = pool.tile([P, DC, 512], F16, tag="yt")
                        nc.sync.dma_start_transpose(
                            yt[:], Yar[512 * s:512 * (s + 1), 0:D])
                        y32 = pool.tile([P, DC, 512], FP32, tag="y32")
                        nc.vector.tensor_copy(y32[:], yt[:])
                        nc.vector.tensor_tensor(xT[:, :, sl], xT[:, :, sl],
                                                y32[:], op=ALU.add)

                layernorm(lp["n2g"], lp["n2b"])

            # ---------- final LN + head ----------
            layernorm(lfg, lfb)
            with (
                tc.tile_pool(name="hdp", bufs=3) as pool,
                tc.tile_pool(name="hdw", bufs=2) as wpl,
                tc.tile_pool(name="hps", bufs=4, space="PSUM") as psum,
            ):
                for vs in range(8):
                    vsl = slice(500 * vs, 500 * (vs + 1))
                    hws = wpl.tile([P, DC, 500], F16, tag="hws")
                    nc.sync.dma_start(
                        hws[:], hw.rearrange("(c p) v -> p c v", p=P)[:, :, vsl])
                    hbs = wpl.tile([1, 500], F16, tag="hbs")
                    nc.sync.dma_start(hbs[:], hb[:, vsl])
                    for t in range(NCH):
                        ps = psum.tile([P, 512], FP32, tag="mm")
                        for c in range(DC):
                            nc.tensor.matmul(
                                ps[:, :500],
                                lhsT=xT16[:, c, 128 * t:128 * (t + 1)],
                                rhs=hws[:, c, :], start=(c == 0), stop=False)
                        nc.tensor.matmul(ps[:, :500], lhsT=ones_row_f16[:],
                                         rhs=hbs[:], start=False, stop=True)
                        osb = pool.tile([P, 500], FP32, tag="osb")
                        nc.any.tensor_copy(osb[:], ps[:, :500])
                        nc.sync.dma_start(out[128 * t:128 * (t + 1), vsl],
                                          osb[:])

    _split_multi_waits(nc)
    return nc


_NC_CACHE = {}


def _get_nc():
    if "nc" not in _NC_CACHE:
        _NC_CACHE["nc"] = build_kernel()
    return _NC_CACHE["nc"]


def kernel(input_ids, params):
    f16 = np.float16
    ids = np.asarray(input_ids).reshape(-1).astype(np.int64)
    emb = np.asarray(params["embedding"], np.float32)
    x0 = emb[ids]
    x0T = np.ascontiguousarray(x0.T)

    def percol(v):
        return np.ascontiguousarray(np.asarray(v, np.float32).reshape(-1, P).T)

    layers = params["layers"]
    base = {"x0T": x0T}
    sh = {c: {} for c in range(NCORE)}
    for l, lp in enumerate(layers):
        in_w = np.asarray(lp["attn_in_w"], np.float32)
        in_b = np.asarray(lp["attn_in_b"], np.float32)
        ow = np.asarray(lp["attn_out_w"], np.float32)
        obv = np.asarray(lp["attn_out_b"], np.float32)
        base[f"l{l}_n1g"] = percol(lp["n1_g"])
        base[f"l{l}_n1b"] = percol(lp["n1_b"])
        base[f"l{l}_n2g"] = percol(lp["n2_g"])
        base[f"l{l}_n2b"] = percol(lp["n2_b"])
        base[f"l{l}_gw"] = np.ascontiguousarray(
            np.asarray(lp["gate_w"], np.float32).T)
        base[f"l{l}_gb"] = np.asarray(lp["gate_b"], np.float32).reshape(1, E)
        base[f"l{l}_ob8"] = percol(obv / NCORE)
        for c in range(NCORE):
            h = c
            qs = slice(HD * h, HD * (h + 1))
            ks = slice(D + HD * h, D + HD * (h + 1))
            vs = slice(2 * D + HD * h, 2 * D + HD * (h + 1))
            sh[c][f"l{l}_wq"] = np.ascontiguousarray(in_w[qs].T).astype(f16)
            sh[c][f"l{l}_bq"] = in_b[qs].reshape(HD, 1).astype(np.float32)
            sh[c][f"l{l}_wk"] = np.ascontiguousarray(in_w[ks].T).astype(f16)
            sh[c][f"l{l}_bk"] = in_b[ks].reshape(HD, 1).astype(np.float32)
            sh[c][f"l{l}_wv"] = np.ascontiguousarray(in_w[vs].T).astype(f16)
            sh[c][f"l{l}_bv"] = in_b[vs].reshape(1, HD).astype(f16)
            sh[c][f"l{l}_ow"] = np.ascontiguousarray(ow[:, qs].T).astype(f16)
            esel = np.zeros((1, E), np.float32)
            esel[0, c] = 1.0
            sh[c][f"l{l}_esel"] = esel
            sh[c][f"l{l}_w1"] = np.asarray(lp["w1"][c], np.float32).astype(f16)
            sh[c][f"l{l}_b1"] = np.ascontiguousarray(
                np.asarray(lp["b1"][c], np.float32).reshape(FC, P).T)
            sh[c][f"l{l}_w2"] = np.asarray(lp["w2"][c], np.float32).astype(f16)
            sh[c][f"l{l}_b2"] = percol(lp["b2"][c])
    base["lfg"] = percol(params["ln_f_g"])
    base["lfb"] = percol(params["ln_f_b"])
    hwT = np.asarray(params["head_w"], np.float32).T
    hbv = np.asarray(params["head_b"], np.float32)
    in_maps = []
    for c in range(NCORE):
        m = dict(base)
        m.update(sh[c])
        m["hw"] = np.ascontiguousarray(hwT[:, VSH * c:VSH * (c + 1)]).astype(f16)
        m["hb"] = hbv[VSH * c:VSH * (c + 1)].reshape(1, VSH).astype(f16)
        in_maps.append(m)

    nc = _get_nc()
    res = run_bass_kernel_spmd(nc, in_maps, core_ids=list(range(NCORE)))
    outs = [res.results[c]["out"] for c in range(NCORE)]
    full = np.concatenate(outs, axis=1)
    B, S_ = np.asarray(input_ids).shape
    return full.reshape(B, S_, -1).astype(np.float32)
